# revision 2
# baseline (speedup 1.0000x reference)
"""Causal multi-head attention on 8 TRN2 NeuronCores.

Sharding: core c -> (batch b = c // 2, head-half hh = c % 2).
Each core computes QKV for its 8 heads over the full sequence of its batch,
causal flash attention, and a partial out-projection using its 512 rows of
w_out. The host sums the two partials per batch (the "all-reduce" of the
tensor-parallel out projection).

v2 layout (all matmul operands bf16):
  KT[c][j] [128, 512]  K^T for head pair j, token chunk c (head 2j rows
                       0:64, head 2j+1 rows 64:128)
  V[t]     [128, 520]  V token-tile t, 8 heads x (64 cols + ones col) for
                       the softmax denominator ("ones trick")
  QT[c][j] [128, 512]  Q^T (even head rows 0:64, odd rows 64:128).  The two
                       S matmuls per k-block contract K=64 each and run
                       CONCURRENTLY via PE row tiling (tile_position (0,0)
                       and (64,0)) -- no zero padding needed.

Schedule: QKV projection for chunk c+1 and the out-projection for chunk c-1
are woven into the attention pair loop of chunk c so TensorE always has
matmul work while ScalarE (exp) drains, and vice versa.

Shapes (hardcoded): B=4, T=2048, D=1024, H=16, HD=64.
"""
import sys

for _p in ('/opt/trn_rl_repo', '/root/.axon_site/_ro/trn_rl_repo'):
    if _p not in sys.path:
        sys.path.insert(0, _p)

import numpy as np

B, T, D = 4, 2048, 1024
H, HD = 16, 64
HPC = H // 2          # heads per core = 8
DPC = HPC * HD        # out-dims per core = 512
N_CORES = 8

_nc_cache = {}


def _build_nc():
    import concourse.bacc as bacc
    import concourse.mybir as mybir
    from concourse.tile import TileContext

    F32 = mybir.dt.float32
    BF16 = mybir.dt.bfloat16
    AF = mybir.ActivationFunctionType
    ALU = mybir.AluOpType

    CH = 512              # token chunk (both proj and attention q-chunk)
    NKB = T // 128        # 16 k-blocks
    NC = T // CH          # 4 chunks
    NDT = D // 128        # 8 input-dim tiles
    VW = HPC * (HD + 1)   # V tile width = 520

    nc = bacc.Bacc('TRN2', target_bir_lowering=False, debug=False)
    xT_d = nc.dram_tensor('xT', [D, T], BF16, kind='ExternalInput')
    wq_d = nc.dram_tensor('wq', [D, DPC], BF16, kind='ExternalInput')
    wk_d = nc.dram_tensor('wk', [D, DPC], BF16, kind='ExternalInput')
    wv_d = nc.dram_tensor('wv', [D, DPC], BF16, kind='ExternalInput')
    wo_d = nc.dram_tensor('wo', [DPC, D], BF16, kind='ExternalInput')
    po_d = nc.dram_tensor('po', [T, D], F32, kind='ExternalOutput')

    with nc.allow_low_precision(reason='bf16 matmuls by design'), \
            TileContext(nc) as tc:
        with (
            tc.tile_pool(name='w', bufs=1) as w_pool,
            tc.tile_pool(name='kt', bufs=1) as kt_pool,
            tc.tile_pool(name='vv', bufs=1) as v_pool,
            tc.tile_pool(name='xs', bufs=2) as x_pool,
            tc.tile_pool(name='qt', bufs=2) as qt_pool,
            tc.tile_pool(name='pt', bufs=3) as pt_pool,
            tc.tile_pool(name='ao', bufs=2) as ao_pool,
            tc.tile_pool(name='osb', bufs=2) as osb_pool,
            tc.tile_pool(name='small', bufs=2) as sm_pool,
            tc.tile_pool(name='ps_s', bufs=2, space='PSUM') as ps_s,
            tc.tile_pool(name='ps_ot', bufs=4, space='PSUM') as ps_ot,
        ):
            WK = [w_pool.tile([128, DPC], BF16, tag=f'wk{d}', name=f'wks{d}')
                  for d in range(NDT)]
            WV = [w_pool.tile([128, DPC], BF16, tag=f'wv{d}', name=f'wvs{d}')
                  for d in range(NDT)]
            WQ = [w_pool.tile([128, DPC], BF16, tag=f'wq{d}', name=f'wqs{d}')
                  for d in range(NDT)]
            WO = [w_pool.tile([128, D], BF16, tag=f'wo{d}', name=f'wos{d}')
                  for d in range(4)]
            KT = [[kt_pool.tile([128, CH], BF16, tag=f'kt{c}_{j}',
                                name=f'kt{c}_{j}') for j in range(4)]
                  for c in range(NC)]
            V = [v_pool.tile([128, VW], BF16, tag=f'v{t}', name=f'v{t}')
                 for t in range(NKB)]

            # pre-warm the ACT exp table and the gpsimd library so the
            # first real exp / affine_select doesn't pay the load
            warm = sm_pool.tile([1, 16], F32, tag='warm', bufs=1)
            warm2 = sm_pool.tile([2, 16], F32, tag='warm2', bufs=1)
            nc.vector.memset(warm[:, :], 0.0)
            nc.scalar.activation(warm[:, :], warm[:, :], AF.Exp)
            nc.gpsimd.affine_select(
                out=warm[:, :], in_=warm[:, :], compare_op=ALU.is_ge,
                fill=0.0, base=0, channel_multiplier=-1, pattern=[[1, 16]])
            nc.gpsimd.partition_broadcast(warm2[:, :], warm[:, :])

            # weight DMAs; WK first (first proj chunk needs them first)
            for d in range(NDT):
                nc.sync.dma_start(WK[d][:, :], wk_d[d*128:(d+1)*128, :])
            for d in range(NDT):
                nc.sync.dma_start(WV[d][:, :], wv_d[d*128:(d+1)*128, :])
            for d in range(NDT):
                nc.sync.dma_start(WQ[d][:, :], wq_d[d*128:(d+1)*128, :])
            for d in range(4):
                nc.sync.dma_start(WO[d][:, :], wo_d[d*128:(d+1)*128, :])
            # ones columns for the softmax-denominator trick
            for t in range(NKB):
                vt3 = V[t].rearrange('p (h c) -> p h c', c=HD + 1)
                nc.gpsimd.memset(vt3[:, :, HD], 1.0)

            xs_tiles = {}
            qt_tiles = {}
            ao_tiles = {}

            def dma_x(c):
                xs = [x_pool.tile([128, CH], BF16, tag=f'x{d}',
                                  name=f'xs{d}_{c}') for d in range(NDT)]
                for d in range(NDT):
                    nc.sync.dma_start(xs[d][:, :],
                                      xT_d[d*128:(d+1)*128, c*CH:(c+1)*CH])
                xs_tiles[c] = xs

            def proj_group(kind, c, i):
                """One 8-matmul projection group for token chunk c."""
                xs = xs_tiles[c]
                if kind == 'K':     # KT[c][i]: out [128 dout, CH tok]
                    pp = ps_ot.tile([128, CH], F32, tag='ot', name='pp')
                    for d in range(NDT):
                        nc.tensor.matmul(
                            pp[:, :], lhsT=WK[d][:, i*128:(i+1)*128],
                            rhs=xs[d][:, :],
                            start=(d == 0), stop=(d == NDT - 1))
                    nc.vector.tensor_copy(KT[c][i][:, :], pp[:, :])
                elif kind == 'V':   # V block c*4+i: out [128 tok, DPC dout]
                    pv = ps_ot.tile([128, DPC], F32, tag='ot', name='pv')
                    for d in range(NDT):
                        nc.tensor.matmul(
                            pv[:, :], lhsT=xs[d][:, i*128:(i+1)*128],
                            rhs=WV[d][:, :],
                            start=(d == 0), stop=(d == NDT - 1))
                    vt3 = V[c*4 + i].rearrange('p (h c) -> p h c', c=HD + 1)
                    nc.vector.tensor_copy(
                        vt3[:, :, 0:HD],
                        pv.rearrange('p (h c) -> p h c', c=HD))
                elif kind == 'Q':   # QT[c][i]: out [128 dout, CH tok]
                    pq = ps_ot.tile([128, CH], F32, tag='ot', name='pq')
                    for d in range(NDT):
                        nc.tensor.matmul(
                            pq[:, :], lhsT=WQ[d][:, i*128:(i+1)*128],
                            rhs=xs[d][:, :],
                            start=(d == 0), stop=(d == NDT - 1))
                    qt = qt_pool.tile([128, CH], BF16, tag=f'qt{i}',
                                      name=f'qt{c}_{i}')
                    nc.vector.tensor_copy(qt[:, :], pq[:, :])
                    qt_tiles.setdefault(c, {})[i] = qt

            def outproj_qt(c, qt_i):
                """Out-projection for query rows [c*CH + qt_i*128 ...)."""
                ao = ao_tiles[c]
                q0 = c * CH
                os = osb_pool.tile([128, D], F32, tag='os', name='os')
                for half in range(2):
                    pj = ps_ot.tile([128, 512], F32, tag='ot', name='pj')
                    for d in range(4):
                        nc.tensor.matmul(
                            pj[:, :],
                            lhsT=ao[d][:, qt_i*128:(qt_i+1)*128],
                            rhs=WO[d][:, half*512:(half+1)*512],
                            start=(d == 0), stop=(d == 3))
                    nc.vector.tensor_copy(
                        os[:, half*512:(half+1)*512], pj[:, :])
                nc.sync.dma_start(
                    po_d[q0+qt_i*128:q0+(qt_i+1)*128, :], os[:, :])

            def attention_j(c, j):
                """Causal attention for head pair j over query chunk c."""
                q0 = c * CH
                nkb = (q0 + CH) // 128
                QTj = qt_tiles[c][j]
                h0, h1 = 2*j, 2*j + 1
                ot0 = ps_ot.tile([HD + 1, CH], F32, tag='ot', name='ot0')
                ot1 = ps_ot.tile([HD + 1, CH], F32, tag='ot', name='ot1')
                pend = None
                for kbp in range(nkb // 2):
                    ka, kb = 2*kbp, 2*kbp + 1
                    lo_a = max(0, ka*128 - q0)
                    lo_b = max(0, kb*128 - q0)
                    s0 = ps_s.tile([128, 2*CH], F32, tag='s', name='s0')
                    s1 = ps_s.tile([128, 2*CH], F32, tag='s', name='s1')
                    pt0 = pt_pool.tile([128, 2*CH], BF16, tag='pt0',
                                       name='pt0')
                    pt1 = pt_pool.tile([128, 2*CH], BF16, tag='pt1',
                                       name='pt1')
                    ksa = KT[ka//4][j][:, (ka % 4)*128:((ka % 4)+1)*128]
                    ksb = KT[kb//4][j][:, (kb % 4)*128:((kb % 4)+1)*128]
                    # S for both heads concurrently: K=64 row tiles at
                    # partition offsets 0 (even head) and 64 (odd head)
                    nc.tensor.matmul(
                        s0[:, lo_a:CH], lhsT=ksa[0:64, :],
                        rhs=QTj[0:64, lo_a:CH], start=True, stop=True)
                    nc.tensor.matmul(
                        s1[:, lo_a:CH], lhsT=ksa[64:128, :],
                        rhs=QTj[64:128, lo_a:CH], start=True, stop=True)
                    nc.tensor.matmul(
                        s0[:, CH+lo_b:2*CH], lhsT=ksb[0:64, :],
                        rhs=QTj[0:64, lo_b:CH], start=True, stop=True)
                    nc.tensor.matmul(
                        s1[:, CH+lo_b:2*CH], lhsT=ksb[64:128, :],
                        rhs=QTj[64:128, lo_b:CH], start=True, stop=True)
                    if pend is not None:
                        for (pk, pl, pc0), ppt in pend:
                            nc.tensor.matmul(
                                ot0[:, pl:CH],
                                lhsT=V[pk][:, (HD+1)*h0:(HD+1)*(h0+1)],
                                rhs=ppt[0][:, pc0+pl:pc0+CH],
                                start=(pk == 0), stop=False)
                            nc.tensor.matmul(
                                ot1[:, pl:CH],
                                lhsT=V[pk][:, (HD+1)*h1:(HD+1)*(h1+1)],
                                rhs=ppt[1][:, pc0+pl:pc0+CH],
                                start=(pk == 0), stop=False)
                    nc.scalar.activation(
                        pt0[:, lo_a:2*CH], s0[:, lo_a:2*CH], AF.Exp)
                    nc.scalar.activation(
                        pt1[:, lo_a:2*CH], s1[:, lo_a:2*CH], AF.Exp)
                    for kx, lox, c0 in ((ka, lo_a, 0), (kb, lo_b, CH)):
                        if kx*128 >= q0:   # causal mask on diagonal block
                            for ptx in (pt0, pt1):
                                nc.gpsimd.affine_select(
                                    out=ptx[:, c0+lox:c0+lox+128],
                                    in_=ptx[:, c0+lox:c0+lox+128],
                                    compare_op=ALU.is_ge, fill=0.0,
                                    base=0, channel_multiplier=-1,
                                    pattern=[[1, 128]])
                    pend = [((ka, lo_a, 0), (pt0, pt1)),
                            ((kb, lo_b, CH), (pt0, pt1))]
                for (pk, pl, pc0), ppt in pend:
                    nc.tensor.matmul(
                        ot0[:, pl:CH],
                        lhsT=V[pk][:, (HD+1)*h0:(HD+1)*(h0+1)],
                        rhs=ppt[0][:, pc0+pl:pc0+CH],
                        start=(pk == 0), stop=(pk == nkb - 1))
                    nc.tensor.matmul(
                        ot1[:, pl:CH],
                        lhsT=V[pk][:, (HD+1)*h1:(HD+1)*(h1+1)],
                        rhs=ppt[1][:, pc0+pl:pc0+CH],
                        start=(pk == 0), stop=(pk == nkb - 1))
                # normalize both heads of the pair
                rp0 = sm_pool.tile([1, CH], F32, tag='rp0', bufs=2)
                rp1 = sm_pool.tile([1, CH], F32, tag='rp1', bufs=2)
                din0 = sm_pool.tile([1, CH], F32, tag='din0', bufs=2)
                din1 = sm_pool.tile([1, CH], F32, tag='din1', bufs=2)
                nc.vector.tensor_copy(din0[:, :], ot0[HD:HD+1, :])
                nc.vector.tensor_copy(din1[:, :], ot1[HD:HD+1, :])
                nc.vector.reciprocal_approx_fast(out=rp0[:, :], in_=din0[:, :])
                nc.vector.reciprocal_approx_fast(out=rp1[:, :], in_=din1[:, :])
                rbs0 = sm_pool.tile([HD, CH], F32, tag='rbs0', bufs=2)
                rbs1 = sm_pool.tile([HD, CH], F32, tag='rbs1', bufs=2)
                nc.gpsimd.partition_broadcast(rbs0[:, :], rp0[:, :])
                nc.gpsimd.partition_broadcast(rbs1[:, :], rp1[:, :])
                ao = ao_tiles[c][j]
                nc.vector.tensor_tensor(
                    out=ao[0:HD, :], in0=ot0[0:HD, :], in1=rbs0[:, :],
                    op=ALU.mult)
                nc.vector.tensor_tensor(
                    out=ao[HD:128, :], in0=ot1[0:HD, :], in1=rbs1[:, :],
                    op=ALU.mult)

            # ---------------- emission schedule ----------------
            dma_x(0)
            dma_x(1)
            for j in range(4):
                proj_group('K', 0, j)
            for tt in range(4):
                proj_group('V', 0, tt)
            for j in range(4):
                proj_group('Q', 0, j)

            # per chunk c: attention(c) woven with proj(c+1) and outproj(c-1)
            for c in range(NC):
                ao_tiles[c] = [ao_pool.tile([128, CH], BF16, tag=f'ao{j}',
                                            name=f'ao{c}_{j}')
                               for j in range(4)]
                if c + 2 < NC:
                    dma_x(c + 2)
                # weave lists: 12 proj groups for chunk c+1, 4 outproj
                # groups for chunk c-1
                weave = []
                if c + 1 < NC:
                    weave += [('K', c+1, i) for i in range(4)]
                    weave += [('V', c+1, i) for i in range(4)]
                    weave += [('Q', c+1, i) for i in range(4)]
                if c - 1 >= 0:
                    weave += [('O', c-1, i) for i in range(4)]
                per_j = (len(weave) + 3) // 4
                for j in range(4):
                    for kind, wc, wi in weave[j*per_j:(j+1)*per_j]:
                        if kind == 'O':
                            outproj_qt(wc, wi)
                        else:
                            proj_group(kind, wc, wi)
                    attention_j(c, j)
                xs_tiles.pop(c, None)
            # remaining out-projections (chunks 2 and 3)
            for qt_i in range(4):
                outproj_qt(2, qt_i)
            for qt_i in range(4):
                outproj_qt(3, qt_i)

    nc.compile()
    return nc


def _get_nc():
    if 'nc' not in _nc_cache:
        _nc_cache['nc'] = _build_nc()
    return _nc_cache['nc']


def kernel(x, w_qkv, w_out, _profile=False):
    import ml_dtypes
    from concourse.bass_utils import run_bass_kernel_spmd

    x = np.asarray(x, dtype=np.float32)
    w_qkv = np.asarray(w_qkv, dtype=np.float32)
    w_out = np.asarray(w_out, dtype=np.float32)

    nc = _get_nc()

    bf16 = ml_dtypes.bfloat16
    scale = np.float32(1.0 / np.sqrt(HD))
    in_maps = []
    for c in range(N_CORES):
        b, hh = c // 2, c % 2
        s, e = hh * DPC, (hh + 1) * DPC
        in_maps.append({
            'xT': np.ascontiguousarray(x[b].T).astype(bf16),
            'wq': np.ascontiguousarray(w_qkv[:, s:e] * scale).astype(bf16),
            'wk': np.ascontiguousarray(w_qkv[:, D+s:D+e]).astype(bf16),
            'wv': np.ascontiguousarray(w_qkv[:, 2*D+s:2*D+e]).astype(bf16),
            'wo': np.ascontiguousarray(w_out[s:e, :]).astype(bf16),
        })

    res = run_bass_kernel_spmd(nc, in_maps, core_ids=list(range(N_CORES)),
                               trace=_profile)
    out = np.empty((B, T, D), np.float32)
    for b in range(B):
        out[b] = res.results[2*b]['po'] + res.results[2*b+1]['po']
    if _profile:
        return out, res
    return out
